# revision 1
# baseline (speedup 1.0000x reference)
"""Trainium2 Bass kernel for empirical CRPS loss (mean reduction).

Problem: forecasts (N=20, B=4, C=1, D=12, H=256, W=256) f32, target (B,C,D,H,W) f32.
CRPS = mean_px [ (1/N) sum_i |x_i - y| - (1/N^2) sum_{i<j} |x_i - x_j| ]

Per pixel, with sorted samples X_(0..19):
  sum_{i<j} |x_i - x_j| = sum_k (2k-19) X_(k)        (order-statistic identity)
  sum_i |x_i - y|       = S_px + 20 y - 2 sum_i min(x_i, y)
Both are linear in per-order-statistic column sums, so the kernel:
  1. converts f32 -> f16 on ScalarE (accum_out gives S and Y for free),
  2. sorts the 20 sample blocks per pixel with a 93-comparator network
     (two optimal 29-CE 10-sorters + Batcher merge, verified by the 0-1
     principle): VectorE tensor_tensor min/max in f16 (2x perf mode), using
     a 21-slot buffer with slot rotation (max -> free slot, min -> in place),
  3. computes min(x_i, y) with a stride-0 broadcast of the target,
  4. reduces each sorted column block / the min blocks to per-partition
     scalars (ScalarE Copy accum_out, DVE tensor_scalar accum),
  5. host combines all per-core [128, cols] partials in float64.

Sharding: pure data parallel over pixels, 8 cores x 393216 px.
"""
import numpy as np

N = 20
P_TOTAL = 4 * 1 * 12 * 256 * 256   # 3145728 pixels
N_CORES = 8
P_CORE = P_TOTAL // N_CORES        # 393216
FB = 1024                          # pixel columns per partition per tile
PT = 128 * FB                      # pixels per tile
NT = P_CORE // PT                  # tiles per core (3)

CVT_CHUNK = 4                      # sample blocks converted per ACT op
MN_CHUNK = 5                       # sample blocks per min/sum chunk

# accumulator columns per tile: 20 sorted col sums, NMN mn sums,
# N/CVT_CHUNK Sx sums, 1 Sy
NMN = N // MN_CHUNK
NCVT = N // CVT_CHUNK
CPT = N + NMN + NCVT + 1

_CACHE = {}

# --- sorting network (93 comparators, verified by 0-1 principle): two
# optimal 29-CE 10-sorters + Batcher odd-even merge(10,10) ------------------
SORT10 = [(4, 9), (3, 8), (2, 7), (1, 6), (0, 5),
          (1, 4), (6, 9), (0, 3), (5, 8),
          (0, 2), (3, 6), (7, 9),
          (0, 1), (2, 4), (5, 7), (8, 9),
          (1, 2), (4, 6), (7, 8), (3, 5),
          (2, 5), (6, 8), (1, 3), (4, 7),
          (2, 3), (6, 7),
          (3, 4), (5, 6),
          (4, 5)]


def _oe_merge(a, b, net):
    n, m = len(a), len(b)
    if n == 0 or m == 0:
        return
    if n == 1 and m == 1:
        net.append((a[0], b[0]))
        return
    _oe_merge(a[::2], b[::2], net)
    _oe_merge(a[1::2], b[1::2], net)
    c = list(a) + list(b)
    for i in range(1, n + m - 1, 2):
        net.append((c[i], c[i + 1]))


def sorting_network(n=N):
    assert n == 20
    net = [(i, j) for (i, j) in SORT10]
    net += [(i + 10, j + 10) for (i, j) in SORT10]
    _oe_merge(list(range(10)), list(range(10, 20)), net)
    return net


def _build_nc(p_core=P_CORE, fb=FB, nt=NT):
    import concourse.bacc as bacc
    import concourse.mybir as mybir
    from concourse.tile import TileContext
    from concourse.ap import AP

    F32 = mybir.dt.float32
    F16 = mybir.dt.float16
    Copy = mybir.ActivationFunctionType.Copy
    FBl, NTl, PTl = fb, nt, 128 * fb
    assert p_core == PTl * NTl
    net = sorting_network(N)

    nc = bacc.Bacc()
    fc = nc.declare_dram_parameter("forecasts", [N, p_core], F32, isOutput=False)
    tg = nc.declare_dram_parameter("target", [p_core], F32, isOutput=False)
    out = nc.declare_dram_parameter("partials", [128, CPT * NTl], F32, isOutput=True)

    with TileContext(nc) as tc:
        with (
            tc.tile_pool(name="io", bufs=2) as iop,
            tc.tile_pool(name="wk", bufs=2) as wkp,
            tc.tile_pool(name="scr", bufs=1) as scrp,
            tc.tile_pool(name="acc", bufs=3) as accp,
        ):
            dumscr = scrp.tile([128, FBl], F16)

            for t in range(NTl):
                p0, c0 = t * PTl, 0
                accbuf = accp.tile([128, CPT], F32, tag="accbuf")
                buf = wkp.tile([128, (N + 6) * FBl], F16, tag="buf")
                yb = wkp.tile([128, FBl], F16, tag="yb")

                # load + convert x in chunks of CVT_CHUNK sample blocks
                for ch in range(N // CVT_CHUNK):
                    i0 = ch * CVT_CHUNK
                    xt = iop.tile([128, CVT_CHUNK * FBl], F32, tag="x32")
                    fap = fc[:, :]
                    src = AP(fap.tensor, i0 * p_core + p0,
                             [[FBl, 128], [p_core, CVT_CHUNK], [1, FBl]])
                    nc.sync.dma_start(
                        xt.rearrange("p (n f) -> p n f", n=CVT_CHUNK), src)
                    nc.scalar.activation(
                        buf[:, i0 * FBl:(i0 + CVT_CHUNK) * FBl], xt, Copy,
                        accum_out=accbuf[:, c0 + N + NMN + ch:c0 + N + NMN + ch + 1])
                yt = iop.tile([128, FBl], F32, tag="y32")
                nc.sync.dma_start(
                    yt, tg[p0:p0 + PTl].rearrange("(p f) -> p f", p=128))
                nc.scalar.activation(yb, yt, Copy,
                                     accum_out=accbuf[:, c0 + CPT - 1:c0 + CPT])

                # Mn: min(x_i, y) in chunks, summed on ScalarE
                yap = yb[:, :]
                for mc in range(NMN):
                    i0 = mc * MN_CHUNK
                    mnscr = wkp.tile([128, MN_CHUNK * FBl], F16, tag="mnscr")
                    yb3 = AP(yap.tensor, yap.offset,
                             [list(yap.ap[0]), [0, MN_CHUNK], list(yap.ap[1])])
                    nc.vector.tensor_tensor(
                        out=mnscr.rearrange("p (n f) -> p n f", n=MN_CHUNK),
                        in0=buf[:, i0 * FBl:(i0 + MN_CHUNK) * FBl]
                            .rearrange("p (n f) -> p n f", n=MN_CHUNK),
                        in1=yb3,
                        op=mybir.AluOpType.min)
                    nc.scalar.activation(
                        mnscr[:, :], mnscr[:, :], Copy,
                        accum_out=accbuf[:, c0 + N + mc:c0 + N + mc + 1])

                # sort the 20 blocks with the comparator network
                # (max -> free slot, min -> in place; zero copies).
                # Independent comparators of one layer whose in0/in1 slots
                # form arithmetic progressions are fused into one multi-dim
                # AP instruction (each tensor has its own stride).
                slot = list(range(N))
                frees = [N, N + 1, N + 2, N + 3, N + 4, N + 5]

                def ap2(base_slot, step, cnt):
                    bap = buf[:, base_slot * FBl:(base_slot + 1) * FBl]
                    if cnt == 1:
                        return bap
                    return AP(bap.tensor, bap.offset,
                              [list(bap.ap[0]), [step * FBl, cnt],
                               list(bap.ap[1])])

                # split network into layers of wire-disjoint comparators
                layers, cur, used = [], [], set()
                for (i, j) in net:
                    if i in used or j in used:
                        layers.append(cur)
                        cur, used = [], set()
                    cur.append((i, j))
                    used.update((i, j))
                layers.append(cur)

                for layer in layers:
                    groups = []
                    for (i, j) in layer:
                        si, sj = slot[i], slot[j]
                        g = groups[-1] if groups else None
                        if g is not None and len(g) >= 1:
                            (i0, j0, s0, t0) = g[0]
                            if len(g) == 1:
                                g.append((i, j, si, sj))
                                continue
                            di, dj = g[1][2] - g[0][2], g[1][3] - g[0][3]
                            if (si - g[-1][2] == di and sj - g[-1][3] == dj
                                    and len(frees) > len(g)):
                                g.append((i, j, si, sj))
                                continue
                        groups.append([(i, j, si, sj)])
                    for g in groups:
                        cnt = len(g)
                        # validate arithmetic progression (pairs always are)
                        if cnt >= 2:
                            di, dj = g[1][2] - g[0][2], g[1][3] - g[0][3]
                            ok = all(g[q][2] - g[q - 1][2] == di and
                                     g[q][3] - g[q - 1][3] == dj
                                     for q in range(1, cnt))
                            if not ok or len(frees) < cnt:
                                # fall back to singles
                                for (i, j, si, sj) in g:
                                    fslot = frees.pop(0)
                                    nc.vector.tensor_tensor(
                                        out=ap2(fslot, 0, 1),
                                        in0=ap2(si, 0, 1), in1=ap2(sj, 0, 1),
                                        op=mybir.AluOpType.max)
                                    nc.vector.tensor_tensor(
                                        out=ap2(si, 0, 1),
                                        in0=ap2(si, 0, 1), in1=ap2(sj, 0, 1),
                                        op=mybir.AluOpType.min)
                                    slot[j] = fslot
                                    frees.append(sj)
                                continue
                        else:
                            di, dj = 0, 0
                        fsl = [frees.pop(0) for _ in range(cnt)]
                        # out slots fsl are consecutive pops; use cnt==1 or
                        # require contiguity by sorting and checking
                        fsl.sort()
                        fo = fsl[1] - fsl[0] if cnt >= 2 else 0
                        if cnt >= 2 and any(fsl[q] - fsl[q - 1] != fo
                                            for q in range(1, cnt)):
                            # non-arithmetic frees: emit singles
                            for (idx, (i, j, si, sj)) in enumerate(g):
                                fslot = fsl[idx]
                                nc.vector.tensor_tensor(
                                    out=ap2(fslot, 0, 1),
                                    in0=ap2(si, 0, 1), in1=ap2(sj, 0, 1),
                                    op=mybir.AluOpType.max)
                                nc.vector.tensor_tensor(
                                    out=ap2(si, 0, 1),
                                    in0=ap2(si, 0, 1), in1=ap2(sj, 0, 1),
                                    op=mybir.AluOpType.min)
                                slot[j] = fslot
                                frees.append(sj)
                            continue
                        s0, j0 = g[0][2], g[0][3]
                        nc.vector.tensor_tensor(
                            out=ap2(fsl[0], fo, cnt),
                            in0=ap2(s0, di, cnt), in1=ap2(j0, dj, cnt),
                            op=mybir.AluOpType.max)
                        nc.vector.tensor_tensor(
                            out=ap2(s0, di, cnt),
                            in0=ap2(s0, di, cnt), in1=ap2(j0, dj, cnt),
                            op=mybir.AluOpType.min)
                        for (idx, (i, j, si, sj)) in enumerate(g):
                            slot[j] = fsl[0] + idx * fo
                            frees.append(sj)
                free = frees[0]

                # per-order-statistic sums (DVE tensor_scalar accum, 4x f16)
                for k in range(N):
                    sk = slot[k]
                    nc.vector.tensor_scalar(
                        out=dumscr,
                        in0=buf[:, sk * FBl:(sk + 1) * FBl],
                        scalar1=1.0, scalar2=None,
                        op0=mybir.AluOpType.mult,
                        op1=mybir.AluOpType.add,
                        accum_out=accbuf[:, c0 + k:c0 + k + 1])

                nc.sync.dma_start(out[:, t * CPT:(t + 1) * CPT], accbuf[:, :])
    nc.compile()
    return nc


def _combine(partials_list):
    """partials cols per tile: [0:20] sorted col sums, [20:20+NMN] mn,
    [20+NMN:20+NMN+NCVT] Sx, [-1] Sy."""
    coef = 2.0 * np.arange(N) - (N - 1)
    tot = 0.0
    for p in partials_list:
        p = np.asarray(p, dtype=np.float64).reshape(128, NT, CPT)
        cs = p[:, :, 0:N].sum(axis=(0, 1))          # per-k column sums
        Mn = p[:, :, N:N + NMN].sum()
        S = p[:, :, N + NMN:N + NMN + NCVT].sum()
        Y = p[:, :, CPT - 1].sum()
        PW = (coef * cs).sum()
        FT = S + N * Y - 2.0 * Mn
        tot += FT / N - PW / (N * N)
    return tot / P_TOTAL


def _run(forecasts, target, trace=False):
    from concourse.bass_utils import run_bass_kernel_spmd

    nc = _CACHE.get("nc")
    if nc is None:
        nc = _build_nc()
        _CACHE["nc"] = nc

    fcf = np.asarray(forecasts, dtype=np.float32).reshape(N, P_TOTAL)
    tgf = np.asarray(target, dtype=np.float32).reshape(P_TOTAL)
    in_maps = []
    for c in range(N_CORES):
        sl = slice(c * P_CORE, (c + 1) * P_CORE)
        in_maps.append({
            "forecasts": np.ascontiguousarray(fcf[:, sl]),
            "target": np.ascontiguousarray(tgf[sl]),
        })
    res = run_bass_kernel_spmd(nc, in_maps, list(range(N_CORES)), trace=trace)
    val = _combine([r["partials"] for r in res.results])
    return np.array(val, dtype=np.float32), res


def kernel(forecasts, target):
    val, _ = _run(forecasts, target)
    return val

